# revision 51
# baseline (speedup 1.0000x reference)
"""Trainium2 Bass kernel for per-frame multi-head attention with partial RoPE.

Problem (hardcoded): b=2, N=4096, dim=512, H=8, DH=64, f=4 frames of n=1024
tokens, ROT_DIM=32 partial rotary, softmax attention per (b, h, frame) block,
then output projection.

Sharding: 8 cores = (batch, frame) pairs. Each core runs all 8 heads for one
1024-token frame - fully independent, no collectives.

Schedule (v8): keeps ACT (64 exp tiles) and PE (~80us of matmuls)
simultaneously busy:
  - Host pre-packs every input so each DMA is one contiguous line per
    partition; per-kc x^T chunks land as separate tiles so the first
    QKV matmuls start as soon as their chunk arrives.
  - A short burst of dependency-free PE warm matmuls (rotating over
    disjoint PSUM ranges) trips the HAM clock gate early; a dummy exp
    pre-loads the ACT spline table.
  - Attention beat = (jc, ih): both heads' S^T matmuls write the two
    512-col halves of one [128,1024] PSUM tile from disjoint 64-row PE
    row-tiles, so they co-run; one exp per beat covers both heads.
    Cross-engine WAR tracking is per-TILE, so each concurrent stream
    owns its own psum tile: 2 rotating S tiles (tag bg), 2 rotating
    extras tiles (tag bgX) for the QKV/V displacement work, 2 sm tiles.
  - All 8 V-projection chunks run in the prologue's DMA/rope window;
    later q/k chunk projections slot between beats on the extras tiles.
  - PV stationary is [128, DH+64]: V plus 64 ones-columns, so the softmax
    denominator arrives pre-broadcast in PSUM rows 64..127; normalization
    is copy + recip + multiply on DVE. i0-major beat order lets the i0
    accumulators normalize mid-pair, so only 2 PSUM banks hold PV state.
  - Each pair's first beat-pair is hoisted into the previous pair's tail;
    the output projection's i0 half is hoisted into pair 3; tail pf
    accumulators reuse freed psum tiles and their bias-copies ride the
    idle ACT engine; output DMAs fire per half-tile.
  - Output is written fp16 and cast to fp32 on the host.
"""

from contextlib import ExitStack

import numpy as np

import concourse.bass as bass
import concourse.tile as tile
from concourse import bacc
from concourse import mybir
from concourse.bass_utils import run_bass_kernel_spmd

F32 = mybir.dt.float32
FP16 = mybir.dt.float16
MM_DT = FP16

B, N, DIM = 2, 4096, 512
H, DH = 8, 64
NF = 4                # frames
NTOK = 1024           # tokens per frame
ROT = 32
SCALE = DH ** -0.5
NCORES = 8

PAIRSWAP = [i ^ 1 for i in range(32)]
N_WARM = 16           # PE pre-warm matmuls (prologue head)


def build_program():
    """Build the single-core Bass/Tile program (SPMD across 8 cores)."""
    nc = bacc.Bacc(trn_type="TRN2", target_bir_lowering=False, debug=False)

    # host-packed inputs: one contiguous line per partition
    xth0_d = nc.dram_tensor("xth0", [128, 4, 512], MM_DT, kind="ExternalInput").ap()
    xth1_d = nc.dram_tensor("xth1", [128, 4, 512], MM_DT, kind="ExternalInput").ap()
    wqk0_d = nc.dram_tensor("wqk0", [128, 4, 256], MM_DT, kind="ExternalInput").ap()
    wv_d = nc.dram_tensor("wv", [128, 4, 512], MM_DT, kind="ExternalInput").ap()
    wrest_d = nc.dram_tensor("wrest", [128, 4, 768], MM_DT, kind="ExternalInput").ap()
    wout_d = nc.dram_tensor("wout", [128, 4, 512], MM_DT, kind="ExternalInput").ap()
    bout_d = nc.dram_tensor("bout", [DIM], F32, kind="ExternalInput").ap()
    csm_d = nc.dram_tensor("csm", [128, 2, NTOK], MM_DT, kind="ExternalInput").ap()
    out_d = nc.dram_tensor("out_t", [DIM, NTOK], MM_DT, kind="ExternalOutput").ap()

    EXP = mybir.ActivationFunctionType.Exp

    with tile.TileContext(nc) as tc, ExitStack() as ctx:
        const = ctx.enter_context(tc.tile_pool(name="const", bufs=1))
        big = ctx.enter_context(tc.tile_pool(name="big", bufs=1))
        work = ctx.enter_context(tc.tile_pool(name="work", bufs=1))
        epool = ctx.enter_context(tc.tile_pool(name="E", bufs=8))
        psum = ctx.enter_context(tc.tile_pool(name="ps", bufs=1, space="PSUM"))

        # PSUM layout (8 banks). Cross-engine WAR tracking is per-TILE, so
        # every independently-flowing stream needs its own tile object:
        # - tag "bg" bufs=2: two rotating [128,1024] S tiles (S write ->
        #   exp read), 4 banks.
        # - tag "bgX" bufs=2: two rotating [128,512] extras tiles for the
        #   QKV/V displacement work, 2 banks.
        # - tag "sm" bufs=2: PV accumulators / pf tiles, 2 banks.
        def bg_tile(name):
            return psum.tile([128, NTOK], F32, tag="bg", name=name, bufs=2)

        def xt_tile(name):
            return psum.tile([128, 512], F32, tag="bgX", name=name, bufs=2)

        def sm_tile(name):
            return psum.tile([128, 512], F32, tag="sm", name=name, bufs=2)

        # ---- PE pre-warm + ACT table pre-load (no data deps) ----
        wtile = const.tile([128, 128], MM_DT, tag="wtile", name="wtile")
        nc.gpsimd.memset(wtile[:], 0.0)
        dume = const.tile([128, 16], MM_DT, tag="dume", name="dume")
        nc.scalar.activation(dume[:], wtile[:, 0:16], EXP)

        def warm(n):
            # rotate over 2 sm slots x 4 disjoint column ranges -> dep
            # distance 8, so warm matmuls issue back-to-back (~107ns cold)
            # and trip the HAM un-throttle quickly.
            for i in range(n):
                wps = sm_tile("wps")
                col = (i % 4) * 128
                nc.tensor.matmul(wps[0:64, col:col + 128], wtile[:, 0:64],
                                 wtile[:], start=True, stop=True)

        warm(N_WARM)

        # ---- input DMA: per-kc x chunks land as separate tiles so the
        # first QKV matmuls start as soon as their chunk arrives.
        wqk0 = const.tile([128, 4, 256], MM_DT, tag="wqk0", name="wqk0_sb")
        nc.sync.dma_start(wqk0[:], wqk0_d)
        # x^T in kc-pair tiles: 2KB partition lines keep DMA efficiency
        # high while still letting the first QKV matmuls start early.
        xTh = [[big.tile([128, 2, 512], MM_DT, tag=f"x{i}{h}", name=f"x{i}{h}")
                for h in range(2)] for i in range(2)]
        for h in range(2):
            nc.sync.dma_start(xTh[0][h][:], xth0_d[:, 2 * h:2 * h + 2, :])
        csm = const.tile([128, 2, NTOK], MM_DT, tag="csm", name="csm_sb")
        nc.sync.dma_start(csm[:], csm_d)
        cosm = csm[:, 0, :]
        sinm = csm[:, 1, :]
        for h in range(2):
            nc.sync.dma_start(xTh[1][h][:], xth1_d[:, 2 * h:2 * h + 2, :])
        wv = const.tile([128, 4, 512], MM_DT, tag="wv", name="wv_sb")
        nc.sync.dma_start(wv[:], wv_d)
        wrest = const.tile([128, 4, 768], MM_DT, tag="wrest", name="wrest_sb")
        nc.sync.dma_start(wrest[:], wrest_d)
        wout = const.tile([128, 4, DIM], MM_DT, tag="wout", name="wout_sb")
        nc.sync.dma_start(wout[:], wout_d)
        bout = const.tile([128, 4], F32, tag="bout", name="bout_sb")
        nc.sync.dma_start(bout[:], bout_d.rearrange("(c p) -> p c", p=128))

        # chunk c (0..7 = q0..q3,k0..k3) -> (weight tile, column offset)
        def wsrc(c):
            if c == 0:
                return wqk0, 0
            if c == 4:
                return wqk0, 128
            if c < 4:
                return wrest, (c - 1) * 128
            return wrest, 384 + (c - 5) * 128

        def xsrc(ih, kc):
            return xTh[ih][kc // 2][:, kc % 2, :]

        # ---- persistent SBUF tiles ----
        qsb = [big.tile([128, NTOK], MM_DT, tag=f"q{c}", name=f"q{c}") for c in range(4)]
        ksb = [big.tile([128, NTOK], MM_DT, tag=f"k{c}", name=f"k{c}") for c in range(4)]
        # V token-major per j-chunk: [128 tok, head, DH + 64 ones cols]
        vsb = [big.tile([128, H, DH + 64], MM_DT, tag=f"v{t}", name=f"v{t}") for t in range(8)]
        obar = [big.tile([128, NTOK], MM_DT, tag=f"ob{c}", name=f"ob{c}") for c in range(4)]
        outsb = [big.tile([128, NTOK], MM_DT, tag=f"os{c}", name=f"os{c}") for c in range(4)]

        for t in range(8):
            nc.gpsimd.memset(vsb[t][:, :, DH:DH + 64], 1.0)

        def rope_half(pqh, dst_half, ih, c0=0, c1=512):
            """Partial rotary straight out of the PSUM half into fp16 SBUF.

            dst = pq*cos + swap(pq)*sin == pq*cos + swap(pq*sin2) where
            sin2 = swap(sin) host-side. c0:c1 selects a token sub-range so
            the prologue can release the first k tokens early.
            """
            w = c1 - c0
            sl = slice(ih * 512 + c0, ih * 512 + c1)
            t = work.tile([128, 512], MM_DT, tag="t16", name="t2", bufs=8)
            nc.vector.tensor_mul(t[:, 0:w], pqh[:, c0:c1], sinm[:, sl])
            p1 = work.tile([128, 512], MM_DT, tag="t16", name="p1", bufs=8)
            nc.vector.tensor_mul(p1[:, 0:w], pqh[:, c0:c1], cosm[:, sl])
            sh = work.tile([128, 512], MM_DT, tag="tsh", name="sh", bufs=4)
            nc.vector.stream_shuffle(sh[:, 0:w], t[:, 0:w], PAIRSWAP)
            nc.vector.tensor_add(dst_half[:, c0:c1], p1[:, 0:w], sh[:, 0:w])

        def qk_mm_half(pq, c, ih):
            wt, co = wsrc(c)
            for kc in range(4):
                nc.tensor.matmul(
                    pq[:],
                    wt[:, kc, co:co + 128],
                    xsrc(ih, kc),
                    start=(kc == 0), stop=(kc == 3),
                )

        def emit_qk(c):
            dst = qsb[c] if c < 4 else ksb[c - 4]
            for ih in range(2):
                pq = xt_tile("pq")
                qk_mm_half(pq, c, ih)
                rope_half(pq[:], dst[:, ih * 512:(ih + 1) * 512], ih)

        def emit_v1(t):
            """V projection for one token chunk t (one extras tile)."""
            pv = xt_tile("pv")
            for kc in range(4):
                nc.tensor.matmul(
                    pv[:],
                    xTh[t // 4][kc // 2][:, kc % 2,
                                         (t % 4) * 128:(t % 4 + 1) * 128],
                    wv[:, kc, :],
                    start=(kc == 0), stop=(kc == 3),
                )
            nc.vector.tensor_copy(
                vsb[t][:, :, 0:DH],
                pv[:].rearrange("p (h d) -> p h d", h=H),
            )

        # extras slotted between attention beats, keyed by (pair, beat idx).
        # Same-beat v1 pairs land on the two rotating extras tiles.
        extras = {
            (0, 2): lambda: (emit_v1(0), emit_v1(1)),
            (0, 3): lambda: (emit_v1(2), emit_v1(3)),
            (0, 4): lambda: (emit_v1(4), emit_v1(5)),
            (0, 5): lambda: (emit_v1(6), emit_v1(7)),
            (0, 6): lambda: emit_qk(1),    # q1
            (0, 9): lambda: emit_qk(5),    # k1 (early enough for the
            (1, 2): lambda: emit_qk(2),    # hoisted next-pair S cycle)
            (1, 5): lambda: emit_qk(6),
            (2, 2): lambda: emit_qk(3),
            (2, 5): lambda: emit_qk(7),
        }

        # i0-major beats, processed as fused cycles of two beats (slots
        # A then B, one [128,2048] exp over both). Cycle 0 of pairs 1..3
        # is hoisted into the previous pair's tail.
        beats = [(jc, 0) for jc in range(8)] + [(jc, 1) for jc in range(8)]
        ets_all = {p: {} for p in range(4)}
        po_all = {p: {} for p in range(4)}

        def s_beat(p, jc, ih, ps):
            for sub in range(2):
                nc.tensor.matmul(
                    ps[:, sub * 512:(sub + 1) * 512],
                    ksb[p][sub * 64:(sub + 1) * 64, jc * 128:(jc + 1) * 128],
                    qsb[p][sub * 64:(sub + 1) * 64, ih * 512:(ih + 1) * 512],
                    start=True, stop=True,
                    tile_position=(sub * 64, 0),
                )

        def s_cycle(p, b0, b1):
            # per-beat S tile + exp; the two rotating bg tiles pipeline
            # (exp of one overlaps S matmuls of the other).
            for b in (b0, b1):
                ps = bg_tile("ps")
                s_beat(p, b[0], b[1], ps)
                et = epool.tile([128, NTOK], MM_DT, tag="E", name="et")
                nc.scalar.activation(et[:], ps[:], EXP)
                ets_all[p][b] = (et, 0)

        def emit_pv(p, beat):
            jc, ih = beat
            et, base = ets_all[p][beat]
            for sub in range(2):
                nc.tensor.matmul(
                    po_all[p][(sub, ih)][:],
                    vsb[jc][:, 2 * p + sub, :],
                    et[:, base + sub * 512:base + (sub + 1) * 512],
                    start=(jc == 0), stop=(jc == 7),
                )

        def norm(p, sub, ih):
            off = sub * 64
            sl = slice(ih * 512, (ih + 1) * 512)
            po = po_all[p][(sub, ih)]
            lcp = work.tile([128, 512], F32, tag="t32", name="lcp", bufs=4)
            if p == 3 and ih == 1:
                # last-pair norms sit on the kernel's critical tail and ACT
                # is idle there: do the denominator partition-shift copy on
                # ScalarE (verified: ACT Identity handles base shifts).
                nc.scalar.activation(
                    lcp[0:64, :], po[64:128, :],
                    mybir.ActivationFunctionType.Identity)
            else:
                nc.vector.tensor_copy(lcp[0:64, :], po[64:128, :])
            rinvb = work.tile([128, 512], F32, tag="t32", name="rinvb", bufs=4)
            nc.vector.reciprocal_approx_fast(rinvb[0:64, :], lcp[0:64, :])
            nc.vector.tensor_mul(
                obar[p][off:off + 64, sl], po[0:DH, :], rinvb[0:64, :])

        def outproj(oc, ih, pf=None, eng="v"):
            if pf is None:
                pf = sm_tile("pf")[:]
            for fc in range(4):
                nc.tensor.matmul(
                    pf,
                    wout[:, fc, oc * 128:(oc + 1) * 128],
                    obar[fc][:, ih * 512:(ih + 1) * 512],
                    start=(fc == 0), stop=(fc == 3),
                )
            sl = slice(ih * 512, (ih + 1) * 512)
            if eng == "s":
                # tail copies ride the idle ACT engine (identity is in the
                # loaded exp table set, so no table reload)
                nc.scalar.activation(
                    outsb[oc][:, sl], pf,
                    mybir.ActivationFunctionType.Identity,
                    bias=bout[:, oc:oc + 1])
            else:
                nc.vector.tensor_scalar_add(
                    outsb[oc][:, sl], pf, bout[:, oc:oc + 1])
            nc.sync.dma_start(
                out_d[oc * 128:(oc + 1) * 128, sl], outsb[oc][:, sl])

        # ---- prologue: q0/k0 i0 halves first (k0's rope in two token
        # slices), then the first S cycle IMMEDIATELY -- the i1-half
        # projections are emitted after it so the in-order PE queue does
        # not make the first exp wait on them. pk01 borrows an sm tile
        # (displaced later by the pair-0 po allocations, after its rope).
        pq00 = xt_tile("pq")
        qk_mm_half(pq00, 0, 0)
        pk00 = bg_tile("pk")
        qk_mm_half(pk00[:, 0:512], 4, 0)
        rope_half(pq00[:], qsb[0][:, 0:512], 0)
        rope_half(pk00[:, 0:512], ksb[0][:, 0:512], 0, 0, 256)
        rope_half(pk00[:, 0:512], ksb[0][:, 0:512], 0, 256, 512)
        s_cycle(0, beats[0], beats[1])
        pq01 = xt_tile("pq")
        qk_mm_half(pq01, 0, 1)
        pk01 = sm_tile("pk")
        qk_mm_half(pk01[:], 4, 1)
        rope_half(pq01[:], qsb[0][:, 512:1024], 1)
        rope_half(pk01[:], ksb[0][:, 512:1024], 1)

        for p in range(4):
            po_all[p][(0, 0)] = sm_tile("po")
            po_all[p][(1, 0)] = sm_tile("po")
            for cyc in range(1, 8):
                b0, b1 = beats[2 * cyc], beats[2 * cyc + 1]
                s_cycle(p, b0, b1)
                for bi in (2 * cyc, 2 * cyc + 1):
                    ex = extras.pop((p, bi), None)
                    if ex is not None:
                        ex()
                if cyc == 1:
                    emit_pv(p, beats[0])
                elif cyc == 5:
                    # finish ih0 PV before norming, then switch po to ih1
                    emit_pv(p, beats[7])
                    norm(p, 0, 0)
                    norm(p, 1, 0)
                    if p == 3:
                        outproj(0, 0)
                        outproj(1, 0)
                        # fully-ready ih0 tail projections fill PE gaps in
                        # cycles 6-7 (extras tiles are free after pair 2)
                        outproj(2, 0, xt_tile("pf")[:])
                        outproj(3, 0, xt_tile("pf")[:])
                    po_all[p][(0, 1)] = sm_tile("po")
                    po_all[p][(1, 1)] = sm_tile("po")
                    emit_pv(p, beats[8])
                else:
                    emit_pv(p, beats[2 * cyc - 3])
                    emit_pv(p, beats[2 * cyc - 2])
            emit_pv(p, beats[13])
            emit_pv(p, beats[14])
            if p == 3:
                # ih1 output projections: the fc0-2 partial accumulations
                # are emitted BEFORE pv(15) so they don't sit behind its
                # et(15) wait in the in-order PE queue; only the fc3 stop
                # matmuls run after the final norms.
                pfA = bg_tile("pf")
                pfB = bg_tile("pf")
                pf1 = [pfA[:, 0:512], pfA[:, 512:1024],
                       pfB[:, 0:512], pfB[:, 512:1024]]
                for oc in range(4):
                    for fc in range(3):
                        nc.tensor.matmul(
                            pf1[oc],
                            wout[:, fc, oc * 128:(oc + 1) * 128],
                            obar[fc][:, 512:1024],
                            start=(fc == 0), stop=False,
                        )
            emit_pv(p, beats[15])
            if p < 3:
                s_cycle(p + 1, beats[0], beats[1])
            norm(p, 0, 1)
            norm(p, 1, 1)

        # ---- output projection tail: fc3 stop matmuls + ACT bias-copies
        # (identity rides the now-idle ACT engine) + per-chunk DMA.
        # Dummy matmuls on the freed extras tiles keep the PE busy while
        # the final norms run on DVE, so HAM doesn't re-throttle the
        # clock for the last projection matmuls.
        for i in range(40):
            wps2 = xt_tile("wps2")
            col = (i % 4) * 128
            nc.tensor.matmul(wps2[0:64, col:col + 128], wtile[:, 0:64],
                             wtile[:], start=True, stop=True)
        for oc in range(4):
            nc.tensor.matmul(
                pf1[oc],
                wout[:, 3, oc * 128:(oc + 1) * 128],
                obar[3][:, 512:1024],
                start=False, stop=True,
            )
            if oc % 2 == 0:
                # alternate the bias-copies between ACT and DVE so the
                # four tail chunks drain in parallel on both engines
                nc.scalar.activation(
                    outsb[oc][:, 512:1024], pf1[oc],
                    mybir.ActivationFunctionType.Identity,
                    bias=bout[:, oc:oc + 1])
            else:
                nc.vector.tensor_scalar_add(
                    outsb[oc][:, 512:1024], pf1[oc], bout[:, oc:oc + 1])
            nc.sync.dma_start(
                out_d[oc * 128:(oc + 1) * 128, 512:1024],
                outsb[oc][:, 512:1024])

    nc.compile()
    return nc


def host_prep(x, W_qkv, W_out, b_out, sin, cos):
    """Build the per-core input tensors (host-side packing, incl. x transpose)."""
    x = np.asarray(x, dtype=np.float32)
    W_qkv = np.asarray(W_qkv, dtype=np.float32).copy()
    W_out = np.ascontiguousarray(np.asarray(W_out, dtype=np.float32))
    b_out = np.ascontiguousarray(np.asarray(b_out, dtype=np.float32))
    sin = np.asarray(sin, dtype=np.float32)
    cos = np.asarray(cos, dtype=np.float32)

    # fold q scaling into W_qkv's q block
    W_qkv[:, 0:H * DH] *= SCALE

    # masked, feature-major cos/sin tiles [128, 1024]
    dloc = np.arange(128) % DH
    sign = np.where(np.arange(128) % 2 == 0, -1.0, 1.0).astype(np.float32)
    cosT = cos.T.astype(np.float32)  # [32, 1024]
    sinT = sin.T.astype(np.float32)
    cosm = np.ones((128, NTOK), dtype=np.float32)
    sinm = np.zeros((128, NTOK), dtype=np.float32)
    rot_rows = dloc < ROT
    cosm[rot_rows] = cosT[dloc[rot_rows]]
    sinm[rot_rows] = sinT[dloc[rot_rows]] * sign[rot_rows][:, None]
    # pre-swap sin rows so the kernel can shuffle after the multiply:
    # swap(pq * sinm2)[d] = pq[d^1] * sinm[d]
    sinm = sinm[[d ^ 1 for d in range(128)], :]

    # weights packed as [128 partitions, 4 kc, cols]
    W4 = np.ascontiguousarray(
        W_qkv.reshape(4, 128, 3 * H * DH).transpose(1, 0, 2)).astype(np.float16)
    wqk0 = np.ascontiguousarray(
        np.concatenate([W4[:, :, 0:128], W4[:, :, 512:640]], axis=2))
    wrest = np.ascontiguousarray(np.concatenate(
        [W4[:, :, 128:512], W4[:, :, 640:1024]], axis=2))
    wv = np.ascontiguousarray(W4[:, :, 1024:1536])
    wout_p = np.ascontiguousarray(
        W_out.reshape(4, 128, DIM).transpose(1, 0, 2)).astype(np.float16)

    csm = np.ascontiguousarray(
        np.stack([cosm, sinm], axis=1)).astype(np.float16)  # [128, 2, 1024]

    shared = {
        "wqk0": wqk0, "wrest": wrest, "wv": wv, "wout": wout_p,
        "bout": b_out, "csm": csm,
    }
    in_maps = []
    for c in range(NCORES):
        bi, fi = c // NF, c % NF
        m = dict(shared)
        xt = x[bi, fi * NTOK:(fi + 1) * NTOK, :].T.astype(np.float16)  # [512, 1024]
        x4 = xt.reshape(4, 128, NTOK).transpose(1, 0, 2)               # [128, 4, 1024]
        m["xth0"] = np.ascontiguousarray(x4[:, :, 0:512])
        m["xth1"] = np.ascontiguousarray(x4[:, :, 512:1024])
        in_maps.append(m)
    return in_maps


_CACHED_NC = None


def kernel(x, W_qkv, W_out, b_out, sin, cos, f=4, **run_kwargs):
    global _CACHED_NC
    assert int(f) == NF
    in_maps = host_prep(x, W_qkv, W_out, b_out, sin, cos)
    if _CACHED_NC is None:
        _CACHED_NC = build_program()
    res = run_bass_kernel_spmd(
        _CACHED_NC, in_maps, core_ids=list(range(NCORES)), **run_kwargs
    )
    out = np.empty((B, N, DIM), dtype=np.float32)
    for c in range(NCORES):
        bi, fi = c // NF, c % NF
        out[bi, fi * NTOK:(fi + 1) * NTOK, :] = res.results[c]["out_t"].T.astype(np.float32)
    if run_kwargs:
        kernel.last_results = res
    return out



# revision 53
# speedup vs baseline: 1.0202x; 1.0202x over previous
"""Trainium2 Bass kernel for per-frame multi-head attention with partial RoPE.

Problem (hardcoded): b=2, N=4096, dim=512, H=8, DH=64, f=4 frames of n=1024
tokens, ROT_DIM=32 partial rotary, softmax attention per (b, h, frame) block,
then output projection.

Sharding: 8 cores = (batch, frame) pairs. Each core runs all 8 heads for one
1024-token frame - fully independent, no collectives.

Schedule (v8): keeps ACT (64 exp tiles) and PE (~80us of matmuls)
simultaneously busy:
  - Host pre-packs every input so each DMA is one contiguous line per
    partition; per-kc x^T chunks land as separate tiles so the first
    QKV matmuls start as soon as their chunk arrives.
  - A short burst of dependency-free PE warm matmuls (rotating over
    disjoint PSUM ranges) trips the HAM clock gate early; a dummy exp
    pre-loads the ACT spline table.
  - Attention beat = (jc, ih): both heads' S^T matmuls write the two
    512-col halves of one [128,1024] PSUM tile from disjoint 64-row PE
    row-tiles, so they co-run; one exp per beat covers both heads.
    Cross-engine WAR tracking is per-TILE, so each concurrent stream
    owns its own psum tile: 2 rotating S tiles (tag bg), 2 rotating
    extras tiles (tag bgX) for the QKV/V displacement work, 2 sm tiles.
  - All 8 V-projection chunks run in the prologue's DMA/rope window;
    later q/k chunk projections slot between beats on the extras tiles.
  - PV stationary is [128, DH+64]: V plus 64 ones-columns, so the softmax
    denominator arrives pre-broadcast in PSUM rows 64..127; normalization
    is copy + recip + multiply on DVE. i0-major beat order lets the i0
    accumulators normalize mid-pair, so only 2 PSUM banks hold PV state.
  - Each pair's first beat-pair is hoisted into the previous pair's tail;
    the output projection's i0 half is hoisted into pair 3; tail pf
    accumulators reuse freed psum tiles and their bias-copies ride the
    idle ACT engine; output DMAs fire per half-tile.
  - Output is written fp16 and cast to fp32 on the host.
"""

from contextlib import ExitStack

import numpy as np

import concourse.bass as bass
import concourse.tile as tile
from concourse import bacc
from concourse import mybir
from concourse.bass_utils import run_bass_kernel_spmd

F32 = mybir.dt.float32
FP16 = mybir.dt.float16
MM_DT = FP16

B, N, DIM = 2, 4096, 512
H, DH = 8, 64
NF = 4                # frames
NTOK = 1024           # tokens per frame
ROT = 32
SCALE = DH ** -0.5
NCORES = 8

PAIRSWAP = [i ^ 1 for i in range(32)]
N_WARM = 16           # PE pre-warm matmuls (prologue head)


def build_program():
    """Build the single-core Bass/Tile program (SPMD across 8 cores)."""
    nc = bacc.Bacc(trn_type="TRN2", target_bir_lowering=False, debug=False)

    # host-packed inputs: one contiguous line per partition
    xth0_d = nc.dram_tensor("xth0", [128, 4, 512], MM_DT, kind="ExternalInput").ap()
    xth1_d = nc.dram_tensor("xth1", [128, 4, 512], MM_DT, kind="ExternalInput").ap()
    wqk0_d = nc.dram_tensor("wqk0", [128, 4, 256], MM_DT, kind="ExternalInput").ap()
    wv_d = nc.dram_tensor("wv", [128, 4, 512], MM_DT, kind="ExternalInput").ap()
    wrest_d = nc.dram_tensor("wrest", [128, 4, 768], MM_DT, kind="ExternalInput").ap()
    wout_d = nc.dram_tensor("wout", [128, 4, 512], MM_DT, kind="ExternalInput").ap()
    bout_d = nc.dram_tensor("bout", [DIM], F32, kind="ExternalInput").ap()
    csm_d = nc.dram_tensor("csm", [128, 2, NTOK], MM_DT, kind="ExternalInput").ap()
    out_d = nc.dram_tensor("out_t", [DIM, NTOK], MM_DT, kind="ExternalOutput").ap()

    EXP = mybir.ActivationFunctionType.Exp

    with tile.TileContext(nc) as tc, ExitStack() as ctx:
        const = ctx.enter_context(tc.tile_pool(name="const", bufs=1))
        big = ctx.enter_context(tc.tile_pool(name="big", bufs=1))
        work = ctx.enter_context(tc.tile_pool(name="work", bufs=1))
        epool = ctx.enter_context(tc.tile_pool(name="E", bufs=8))
        psum = ctx.enter_context(tc.tile_pool(name="ps", bufs=1, space="PSUM"))

        # PSUM layout (8 banks). Cross-engine WAR tracking is per-TILE, so
        # every independently-flowing stream needs its own tile object:
        # - tag "bg" bufs=2: two rotating [128,1024] S tiles (S write ->
        #   exp read), 4 banks.
        # - tag "bgX" bufs=2: two rotating [128,512] extras tiles for the
        #   QKV/V displacement work, 2 banks.
        # - tag "sm" bufs=2: PV accumulators / pf tiles, 2 banks.
        def bg_tile(name):
            return psum.tile([128, NTOK], F32, tag="bg", name=name, bufs=2)

        def xt_tile(name):
            return psum.tile([128, 512], F32, tag="bgX", name=name, bufs=2)

        def sm_tile(name):
            return psum.tile([128, 512], F32, tag="sm", name=name, bufs=2)

        # ---- PE pre-warm + ACT table pre-load (no data deps) ----
        wtile = const.tile([128, 128], MM_DT, tag="wtile", name="wtile")
        nc.gpsimd.memset(wtile[:], 0.0)
        dume = const.tile([128, 16], MM_DT, tag="dume", name="dume")
        nc.scalar.activation(dume[:], wtile[:, 0:16], EXP)

        def warm(n):
            # rotate over 2 sm slots x 4 disjoint column ranges -> dep
            # distance 8, so warm matmuls issue back-to-back (~107ns cold)
            # and trip the HAM un-throttle quickly.
            for i in range(n):
                wps = sm_tile("wps")
                col = (i % 4) * 128
                nc.tensor.matmul(wps[0:64, col:col + 128], wtile[:, 0:64],
                                 wtile[:], start=True, stop=True)

        warm(N_WARM)

        # ---- input DMA: per-kc x chunks land as separate tiles so the
        # first QKV matmuls start as soon as their chunk arrives.
        wqk0 = const.tile([128, 4, 256], MM_DT, tag="wqk0", name="wqk0_sb")
        nc.sync.dma_start(wqk0[:], wqk0_d)
        # x^T in kc-pair tiles: 2KB partition lines keep DMA efficiency
        # high while still letting the first QKV matmuls start early.
        xTh = [[big.tile([128, 2, 512], MM_DT, tag=f"x{i}{h}", name=f"x{i}{h}")
                for h in range(2)] for i in range(2)]
        for h in range(2):
            nc.sync.dma_start(xTh[0][h][:], xth0_d[:, 2 * h:2 * h + 2, :])
        csm = const.tile([128, 2, NTOK], MM_DT, tag="csm", name="csm_sb")
        nc.sync.dma_start(csm[:], csm_d)
        cosm = csm[:, 0, :]
        sinm = csm[:, 1, :]
        for h in range(2):
            nc.sync.dma_start(xTh[1][h][:], xth1_d[:, 2 * h:2 * h + 2, :])
        wv = const.tile([128, 4, 512], MM_DT, tag="wv", name="wv_sb")
        nc.sync.dma_start(wv[:], wv_d)
        wrest = const.tile([128, 4, 768], MM_DT, tag="wrest", name="wrest_sb")
        nc.sync.dma_start(wrest[:], wrest_d)
        wout = const.tile([128, 4, DIM], MM_DT, tag="wout", name="wout_sb")
        nc.sync.dma_start(wout[:], wout_d)
        bout = const.tile([128, 4], F32, tag="bout", name="bout_sb")
        nc.sync.dma_start(bout[:], bout_d.rearrange("(c p) -> p c", p=128))

        # chunk c (0..7 = q0..q3,k0..k3) -> (weight tile, column offset)
        def wsrc(c):
            if c == 0:
                return wqk0, 0
            if c == 4:
                return wqk0, 128
            if c < 4:
                return wrest, (c - 1) * 128
            return wrest, 384 + (c - 5) * 128

        def xsrc(ih, kc):
            return xTh[ih][kc // 2][:, kc % 2, :]

        # ---- persistent SBUF tiles ----
        qsb = [big.tile([128, NTOK], MM_DT, tag=f"q{c}", name=f"q{c}") for c in range(4)]
        ksb = [big.tile([128, NTOK], MM_DT, tag=f"k{c}", name=f"k{c}") for c in range(4)]
        # V token-major per j-chunk: [128 tok, head, DH + 64 ones cols]
        vsb = [big.tile([128, H, DH + 64], MM_DT, tag=f"v{t}", name=f"v{t}") for t in range(8)]
        obar = [big.tile([128, NTOK], MM_DT, tag=f"ob{c}", name=f"ob{c}") for c in range(4)]
        outsb = [big.tile([128, NTOK], MM_DT, tag=f"os{c}", name=f"os{c}") for c in range(4)]

        for t in range(8):
            nc.gpsimd.memset(vsb[t][:, :, DH:DH + 64], 1.0)

        def rope_half(pqh, dst_half, ih, c0=0, c1=512):
            """Partial rotary straight out of the PSUM half into fp16 SBUF.

            dst = pq*cos + swap(pq)*sin == pq*cos + swap(pq*sin2) where
            sin2 = swap(sin) host-side. c0:c1 selects a token sub-range so
            the prologue can release the first k tokens early.
            """
            w = c1 - c0
            sl = slice(ih * 512 + c0, ih * 512 + c1)
            t = work.tile([128, 512], MM_DT, tag="t16", name="t2", bufs=8)
            nc.vector.tensor_mul(t[:, 0:w], pqh[:, c0:c1], sinm[:, sl])
            p1 = work.tile([128, 512], MM_DT, tag="t16", name="p1", bufs=8)
            nc.vector.tensor_mul(p1[:, 0:w], pqh[:, c0:c1], cosm[:, sl])
            sh = work.tile([128, 512], MM_DT, tag="tsh", name="sh", bufs=4)
            nc.vector.stream_shuffle(sh[:, 0:w], t[:, 0:w], PAIRSWAP)
            nc.vector.tensor_add(dst_half[:, c0:c1], p1[:, 0:w], sh[:, 0:w])

        def qk_mm_half(pq, c, ih):
            wt, co = wsrc(c)
            for kc in range(4):
                nc.tensor.matmul(
                    pq[:],
                    wt[:, kc, co:co + 128],
                    xsrc(ih, kc),
                    start=(kc == 0), stop=(kc == 3),
                )

        def emit_qk(c):
            dst = qsb[c] if c < 4 else ksb[c - 4]
            for ih in range(2):
                pq = xt_tile("pq")
                qk_mm_half(pq, c, ih)
                rope_half(pq[:], dst[:, ih * 512:(ih + 1) * 512], ih)

        def emit_v1(t):
            """V projection for one token chunk t (one extras tile).

            The PSUM->SBUF cast rides ACT Identity: it fills the exp
            stream's pair-0 gaps instead of loading the DVE, whose
            congestion otherwise delays pv(0) and the q1/k1 ropes.
            """
            pv = xt_tile("pv")
            for kc in range(4):
                nc.tensor.matmul(
                    pv[:],
                    xTh[t // 4][kc // 2][:, kc % 2,
                                         (t % 4) * 128:(t % 4 + 1) * 128],
                    wv[:, kc, :],
                    start=(kc == 0), stop=(kc == 3),
                )
            nc.scalar.activation(
                vsb[t][:, :, 0:DH],
                pv[:].rearrange("p (h d) -> p h d", h=H),
                mybir.ActivationFunctionType.Identity,
            )

        # extras slotted between attention beats, keyed by (pair, beat idx).
        # Same-beat v1 pairs land on the two rotating extras tiles.
        extras = {
            (0, 2): lambda: (emit_v1(0), emit_v1(1)),
            (0, 3): lambda: (emit_v1(2), emit_v1(3)),
            (0, 4): lambda: (emit_v1(4), emit_v1(5)),
            (0, 5): lambda: (emit_v1(6), emit_v1(7)),
            (0, 6): lambda: emit_qk(1),    # q1
            (0, 9): lambda: emit_qk(5),    # k1 (early enough for the
            (1, 2): lambda: emit_qk(2),    # hoisted next-pair S cycle)
            (1, 5): lambda: emit_qk(6),
            (2, 2): lambda: emit_qk(3),
            (2, 5): lambda: emit_qk(7),
        }

        # i0-major beats, processed as fused cycles of two beats (slots
        # A then B, one [128,2048] exp over both). Cycle 0 of pairs 1..3
        # is hoisted into the previous pair's tail.
        beats = [(jc, 0) for jc in range(8)] + [(jc, 1) for jc in range(8)]
        ets_all = {p: {} for p in range(4)}
        po_all = {p: {} for p in range(4)}

        def s_beat(p, jc, ih, ps):
            for sub in range(2):
                nc.tensor.matmul(
                    ps[:, sub * 512:(sub + 1) * 512],
                    ksb[p][sub * 64:(sub + 1) * 64, jc * 128:(jc + 1) * 128],
                    qsb[p][sub * 64:(sub + 1) * 64, ih * 512:(ih + 1) * 512],
                    start=True, stop=True,
                    tile_position=(sub * 64, 0),
                )

        def s_cycle(p, b0, b1):
            # per-beat S tile + exp; the two rotating bg tiles pipeline
            # (exp of one overlaps S matmuls of the other).
            for b in (b0, b1):
                ps = bg_tile("ps")
                s_beat(p, b[0], b[1], ps)
                et = epool.tile([128, NTOK], MM_DT, tag="E", name="et")
                nc.scalar.activation(et[:], ps[:], EXP)
                ets_all[p][b] = (et, 0)

        def emit_pv(p, beat):
            jc, ih = beat
            et, base = ets_all[p][beat]
            for sub in range(2):
                nc.tensor.matmul(
                    po_all[p][(sub, ih)][:],
                    vsb[jc][:, 2 * p + sub, :],
                    et[:, base + sub * 512:base + (sub + 1) * 512],
                    start=(jc == 0), stop=(jc == 7),
                )

        def norm(p, sub, ih):
            off = sub * 64
            sl = slice(ih * 512, (ih + 1) * 512)
            po = po_all[p][(sub, ih)]
            lcp = work.tile([128, 512], F32, tag="t32", name="lcp", bufs=4)
            nc.vector.tensor_copy(lcp[0:64, :], po[64:128, :])
            rinvb = work.tile([128, 512], F32, tag="t32", name="rinvb", bufs=4)
            nc.vector.reciprocal_approx_fast(rinvb[0:64, :], lcp[0:64, :])
            nc.vector.tensor_mul(
                obar[p][off:off + 64, sl], po[0:DH, :], rinvb[0:64, :])

        def outproj(oc, ih, pf=None, eng="v"):
            if pf is None:
                pf = sm_tile("pf")[:]
            for fc in range(4):
                nc.tensor.matmul(
                    pf,
                    wout[:, fc, oc * 128:(oc + 1) * 128],
                    obar[fc][:, ih * 512:(ih + 1) * 512],
                    start=(fc == 0), stop=(fc == 3),
                )
            sl = slice(ih * 512, (ih + 1) * 512)
            if eng == "s":
                # tail copies ride the idle ACT engine (identity is in the
                # loaded exp table set, so no table reload)
                nc.scalar.activation(
                    outsb[oc][:, sl], pf,
                    mybir.ActivationFunctionType.Identity,
                    bias=bout[:, oc:oc + 1])
            else:
                nc.vector.tensor_scalar_add(
                    outsb[oc][:, sl], pf, bout[:, oc:oc + 1])
            nc.sync.dma_start(
                out_d[oc * 128:(oc + 1) * 128, sl], outsb[oc][:, sl])

        # ---- prologue: q0/k0 i0 halves first (k0's rope in two token
        # slices), then the first S cycle IMMEDIATELY -- the i1-half
        # projections are emitted after it so the in-order PE queue does
        # not make the first exp wait on them. pk01 borrows an sm tile
        # (displaced later by the pair-0 po allocations, after its rope).
        pq00 = xt_tile("pq")
        qk_mm_half(pq00, 0, 0)
        pk00 = bg_tile("pk")
        qk_mm_half(pk00[:, 0:512], 4, 0)
        rope_half(pq00[:], qsb[0][:, 0:512], 0)
        rope_half(pk00[:, 0:512], ksb[0][:, 0:512], 0, 0, 256)
        rope_half(pk00[:, 0:512], ksb[0][:, 0:512], 0, 256, 512)
        s_cycle(0, beats[0], beats[1])
        pq01 = xt_tile("pq")
        qk_mm_half(pq01, 0, 1)
        pk01 = sm_tile("pk")
        qk_mm_half(pk01[:], 4, 1)
        rope_half(pq01[:], qsb[0][:, 512:1024], 1)
        rope_half(pk01[:], ksb[0][:, 512:1024], 1)

        for p in range(4):
            po_all[p][(0, 0)] = sm_tile("po")
            po_all[p][(1, 0)] = sm_tile("po")
            for cyc in range(1, 8):
                b0, b1 = beats[2 * cyc], beats[2 * cyc + 1]
                s_cycle(p, b0, b1)
                for bi in (2 * cyc, 2 * cyc + 1):
                    ex = extras.pop((p, bi), None)
                    if ex is not None:
                        ex()
                if cyc == 1:
                    emit_pv(p, beats[0])
                elif cyc == 5:
                    # finish ih0 PV before norming, then switch po to ih1
                    emit_pv(p, beats[7])
                    norm(p, 0, 0)
                    norm(p, 1, 0)
                    if p == 3:
                        outproj(0, 0)
                        outproj(1, 0)
                        # fully-ready ih0 tail projections fill PE gaps in
                        # cycles 6-7 (extras tiles are free after pair 2)
                        outproj(2, 0, xt_tile("pf")[:])
                        outproj(3, 0, xt_tile("pf")[:])
                    po_all[p][(0, 1)] = sm_tile("po")
                    po_all[p][(1, 1)] = sm_tile("po")
                    emit_pv(p, beats[8])
                else:
                    emit_pv(p, beats[2 * cyc - 3])
                    emit_pv(p, beats[2 * cyc - 2])
            emit_pv(p, beats[13])
            emit_pv(p, beats[14])
            if p == 3:
                # ih1 output projections: the fc0-2 partial accumulations
                # are emitted BEFORE pv(15) so they don't sit behind its
                # et(15) wait in the in-order PE queue; only the fc3 stop
                # matmuls run after the final norms.
                pfA = bg_tile("pf")
                pfB = bg_tile("pf")
                pf1 = [pfA[:, 0:512], pfA[:, 512:1024],
                       pfB[:, 0:512], pfB[:, 512:1024]]
                for oc in range(4):
                    for fc in range(3):
                        nc.tensor.matmul(
                            pf1[oc],
                            wout[:, fc, oc * 128:(oc + 1) * 128],
                            obar[fc][:, 512:1024],
                            start=(fc == 0), stop=False,
                        )
            emit_pv(p, beats[15])
            if p < 3:
                s_cycle(p + 1, beats[0], beats[1])
            norm(p, 0, 1)
            norm(p, 1, 1)

        # ---- output projection tail: fc3 stop matmuls + ACT bias-copies
        # (identity rides the now-idle ACT engine) + per-chunk DMA.
        # Dummy matmuls on the freed extras tiles keep the PE busy while
        # the final norms run on DVE, so HAM doesn't re-throttle the
        # clock for the last projection matmuls.
        for i in range(24):
            wps2 = xt_tile("wps2")
            col = (i % 4) * 128
            nc.tensor.matmul(wps2[0:64, col:col + 128], wtile[:, 0:64],
                             wtile[:], start=True, stop=True)
        for oc in range(4):
            nc.tensor.matmul(
                pf1[oc],
                wout[:, 3, oc * 128:(oc + 1) * 128],
                obar[3][:, 512:1024],
                start=False, stop=True,
            )
            nc.scalar.activation(
                outsb[oc][:, 512:1024], pf1[oc],
                mybir.ActivationFunctionType.Identity,
                bias=bout[:, oc:oc + 1])
            nc.sync.dma_start(
                out_d[oc * 128:(oc + 1) * 128, 512:1024],
                outsb[oc][:, 512:1024])

    nc.compile()
    return nc


def host_prep(x, W_qkv, W_out, b_out, sin, cos):
    """Build the per-core input tensors (host-side packing, incl. x transpose)."""
    x = np.asarray(x, dtype=np.float32)
    W_qkv = np.asarray(W_qkv, dtype=np.float32).copy()
    W_out = np.ascontiguousarray(np.asarray(W_out, dtype=np.float32))
    b_out = np.ascontiguousarray(np.asarray(b_out, dtype=np.float32))
    sin = np.asarray(sin, dtype=np.float32)
    cos = np.asarray(cos, dtype=np.float32)

    # fold q scaling into W_qkv's q block
    W_qkv[:, 0:H * DH] *= SCALE

    # masked, feature-major cos/sin tiles [128, 1024]
    dloc = np.arange(128) % DH
    sign = np.where(np.arange(128) % 2 == 0, -1.0, 1.0).astype(np.float32)
    cosT = cos.T.astype(np.float32)  # [32, 1024]
    sinT = sin.T.astype(np.float32)
    cosm = np.ones((128, NTOK), dtype=np.float32)
    sinm = np.zeros((128, NTOK), dtype=np.float32)
    rot_rows = dloc < ROT
    cosm[rot_rows] = cosT[dloc[rot_rows]]
    sinm[rot_rows] = sinT[dloc[rot_rows]] * sign[rot_rows][:, None]
    # pre-swap sin rows so the kernel can shuffle after the multiply:
    # swap(pq * sinm2)[d] = pq[d^1] * sinm[d]
    sinm = sinm[[d ^ 1 for d in range(128)], :]

    # weights packed as [128 partitions, 4 kc, cols]
    W4 = np.ascontiguousarray(
        W_qkv.reshape(4, 128, 3 * H * DH).transpose(1, 0, 2)).astype(np.float16)
    wqk0 = np.ascontiguousarray(
        np.concatenate([W4[:, :, 0:128], W4[:, :, 512:640]], axis=2))
    wrest = np.ascontiguousarray(np.concatenate(
        [W4[:, :, 128:512], W4[:, :, 640:1024]], axis=2))
    wv = np.ascontiguousarray(W4[:, :, 1024:1536])
    wout_p = np.ascontiguousarray(
        W_out.reshape(4, 128, DIM).transpose(1, 0, 2)).astype(np.float16)

    csm = np.ascontiguousarray(
        np.stack([cosm, sinm], axis=1)).astype(np.float16)  # [128, 2, 1024]

    shared = {
        "wqk0": wqk0, "wrest": wrest, "wv": wv, "wout": wout_p,
        "bout": b_out, "csm": csm,
    }
    in_maps = []
    for c in range(NCORES):
        bi, fi = c // NF, c % NF
        m = dict(shared)
        xt = x[bi, fi * NTOK:(fi + 1) * NTOK, :].T.astype(np.float16)  # [512, 1024]
        x4 = xt.reshape(4, 128, NTOK).transpose(1, 0, 2)               # [128, 4, 1024]
        m["xth0"] = np.ascontiguousarray(x4[:, :, 0:512])
        m["xth1"] = np.ascontiguousarray(x4[:, :, 512:1024])
        in_maps.append(m)
    return in_maps


_CACHED_NC = None


def kernel(x, W_qkv, W_out, b_out, sin, cos, f=4, **run_kwargs):
    global _CACHED_NC
    assert int(f) == NF
    in_maps = host_prep(x, W_qkv, W_out, b_out, sin, cos)
    if _CACHED_NC is None:
        _CACHED_NC = build_program()
    res = run_bass_kernel_spmd(
        _CACHED_NC, in_maps, core_ids=list(range(NCORES)), **run_kwargs
    )
    out = np.empty((B, N, DIM), dtype=np.float32)
    for c in range(NCORES):
        bi, fi = c // NF, c % NF
        out[bi, fi * NTOK:(fi + 1) * NTOK, :] = res.results[c]["out_t"].T.astype(np.float32)
    if run_kwargs:
        kernel.last_results = res
    return out

